# revision 58
# baseline (speedup 1.0000x reference)
"""InternLM3 attention block on 8 Trainium2 NeuronCores (Bass/Tile), v2.

Sharding (tensor-parallel over heads, using the GQA structure):
  core c owns Q heads [4c,4c+4) and KV head c; per-core fused pipeline over
  512-token blocks; attention outputs AllGathered in 8 chunks; each core
  computes its 512-column slice of the output projection.

v2 performance structure (vs v1):
  - QKV + output-projection GEMMs run in bf16 (halved LDWEIGHTS cost and
    SBUF footprint; fp32 PSUM accumulation keeps error ~1e-3).
  - Single merged loop: the output projection of chunk ch-1 is interleaved
    into the attention of chunk ch via a work feeder, so the PE array has
    independent GEMM work to fill softmax dependency bubbles and there is
    no serial projection tail.
  - QKV as six per-output chains over the full contraction (2 PSUM banks
    instead of 6), leaving banks for scores(2)/pv(1)/outproj(2).
  - Causal diagonal tiles restrict matmul/exp/mask work to the unmasked
    query range (free-dim subrange), cutting ~30% of attention rows.
  - RoPE reads Q/K straight from PSUM with partition-shifted multiplies
    (no staging copies); v transposed via PE identity matmul.
  - Denominator: DVE accumulation + ones-matmul, reciprocal_approx_fast,
    gpsimd partition_broadcast.
  - DMA queues split: x/weights/at on sync, ao/out stores on gpsimd.
"""

import math
import os
import sys

if "/opt/trn_rl_repo" not in sys.path:
    sys.path.insert(0, "/opt/trn_rl_repo")

import numpy as np
import ml_dtypes

import concourse.bass as bass
import concourse.mybir as mybir
import concourse.tile as tile
from concourse import bacc
from concourse import bass_utils

# ---- problem constants (hardcoded per harness contract) ----
HIDDEN = 4096
N_HEADS = 32
N_KV_HEADS = 8
HEAD_DIM = 128
ROPE_THETA = 10000.0
B, S = 2, 2048
NCORES = 8

P = 128
TQ = 512                      # token block
NB = S // TQ                  # 4 blocks per batch
KT = HIDDEN // P              # 32 contraction tiles
QH = N_HEADS // NCORES        # 4 q-heads per core
HG = QH * HEAD_DIM            # 512 = head-group width per core
NCHUNK = B * NB               # 8 allgather chunks
TOK = B * S                   # 4096 tokens

f32 = mybir.dt.float32
f32r = mybir.dt.float32r
bf16 = mybir.dt.bfloat16
BF = ml_dtypes.bfloat16


def _build_module(with_collectives=True):
    nc = bacc.Bacc("TRN2", target_bir_lowering=False, debug=False,
                   num_devices=NCORES)
    nc._skip_collectives = not with_collectives

    xT = nc.dram_tensor("xT", [HIDDEN, TOK], bf16, kind="ExternalInput").ap()
    wqT = nc.dram_tensor("wqT", [HIDDEN, HG], bf16, kind="ExternalInput").ap()
    wkT = nc.dram_tensor("wkT", [HIDDEN, HEAD_DIM], bf16,
                         kind="ExternalInput").ap()
    wvT = nc.dram_tensor("wvT", [HIDDEN, HEAD_DIM], bf16,
                         kind="ExternalInput").ap()
    woT = nc.dram_tensor("woT", [HIDDEN, HG], bf16, kind="ExternalInput").ap()
    cosIn = nc.dram_tensor("cosIn", [P, S], bf16, kind="ExternalInput").ap()
    ssinIn = nc.dram_tensor("ssinIn", [P, S], bf16, kind="ExternalInput").ap()
    masksIn = nc.dram_tensor("masksIn", [P, 4 * TQ], bf16,
                             kind="ExternalInput").ap()
    identIn = nc.dram_tensor("identIn", [P, P], f32, kind="ExternalInput").ap()
    onesIn = nc.dram_tensor("onesIn", [P, 1], f32r, kind="ExternalInput").ap()
    outT = nc.dram_tensor("outT", [HG, TOK], f32, kind="ExternalOutput").ap()

    ag_in = [
        nc.dram_tensor(f"ag_in{i}", [HG, TQ], bf16, kind="Internal").ap()
        for i in range(NCHUNK - 1)
    ]
    ag_out = [
        nc.dram_tensor(f"ag_out{i}", [HIDDEN, TQ], bf16, kind="Internal",
                       addr_space="Shared").ap()
        for i in range(NCHUNK - 1)
    ]
    # last chunk gathers per head so the final projection can start before
    # the whole block's attention (and its laggard cores) finish
    agt_in = [
        nc.dram_tensor(f"agt_in{h}", [P, TQ], bf16, kind="Internal").ap()
        for h in range(QH)
    ]
    agt_out = [
        nc.dram_tensor(f"agt_out{h}", [NCORES * P, TQ], bf16,
                       kind="Internal", addr_space="Shared").ap()
        for h in range(QH)
    ]

    with tile.TileContext(nc) as tc:
        _body(tc, nc, xT, wqT, wkT, wvT, woT, cosIn, ssinIn, masksIn, identIn,
              onesIn, outT, ag_in, ag_out, agt_in, agt_out)
    nc.compile()
    return nc


def _body(tc, nc, xT, wqT, wkT, wvT, woT, cosIn, ssinIn, masksIn, identIn,
          onesIn, outT, ag_in, ag_out, agt_in, agt_out):
    AF = mybir.ActivationFunctionType
    OP = mybir.AluOpType

    with (
        tc.tile_pool(name="wpool", bufs=1) as wpool,
        tc.tile_pool(name="xpool", bufs=4) as xpool,
        tc.tile_pool(name="kvpool", bufs=1) as kvpool,
        tc.tile_pool(name="qpool", bufs=1) as qpool,
        tc.tile_pool(name="stage", bufs=1) as stage,
        tc.tile_pool(name="epool", bufs=2) as epool,
        tc.tile_pool(name="aux", bufs=2) as aux,
        tc.tile_pool(name="atpool", bufs=3) as atpool,
        tc.tile_pool(name="obpool", bufs=2) as obpool,
        tc.tile_pool(name="pq", bufs=3, space="PSUM") as pq,
        tc.tile_pool(name="ppv", bufs=1, space="PSUM") as ppv,
        tc.tile_pool(name="pst", bufs=2, space="PSUM") as pst,
        tc.tile_pool(name="pop", bufs=1, space="PSUM") as pop,
    ):
        # ---- resident weight/const tiles (DMAs issued interleaved below) --
        wq_sb = [wpool.tile([P, KT, P], bf16, tag=f"wq{j}", name=f"wq{j}")
                 for j in range(QH)]
        wk_sb = wpool.tile([P, KT, P], bf16, tag="wk")
        wv_sb = wpool.tile([P, KT, P], bf16, tag="wv")
        wo_sb = wpool.tile([P, KT, HG], bf16, tag="wo")
        cos_sb = wpool.tile([P, S], bf16, tag="cos")
        sin_sb = wpool.tile([P, S], bf16, tag="sin")
        mask_sb = wpool.tile([P, 4, TQ], bf16, tag="mask")
        id_sb = wpool.tile([P, P], f32, tag="ident")
        ones_sb = wpool.tile([P, 1], f32r, tag="ones")

        def w_dma(t, src, j=None):
            if j is None:
                nc.sync.dma_start(t[:], src)
            else:
                nc.sync.dma_start(
                    t[:], src[:, j * P:(j + 1) * P].rearrange(
                        "(ko p) m -> p ko m", p=P))

        def issue_x(b, n):
            """Issue the 4 quarter DMAs of x for token block (b, n)."""
            tok0 = b * S + n * TQ
            tiles = []
            for qt in range(4):
                t = xpool.tile([P, 8, TQ], bf16, tag="xq",
                               name=f"xq{b}_{n}_{qt}")
                nc.sync.dma_start(
                    t[:],
                    xT[qt * 8 * P:(qt + 1) * 8 * P, tok0:tok0 + TQ].rearrange(
                        "(ko p) t -> p ko t", p=P))
                tiles.append(t)
            return tiles

        # startup order: wq0, x(0,0) quarters interleaved with wq1-3, then
        # the rest. Keeps the first QKV chain start at ~6us.
        w_dma(wq_sb[0], wqT, 0)
        x0_tiles = issue_x(0, 0)
        for j in range(1, QH):
            w_dma(wq_sb[j], wqT, j)
        w_dma(wk_sb, wkT, 0)
        w_dma(wv_sb, wvT, 0)
        nc.sync.dma_start(cos_sb[:], cosIn)
        nc.sync.dma_start(sin_sb[:], ssinIn)
        nc.sync.dma_start(mask_sb[:], masksIn.rearrange("p (r t) -> p r t", r=4))
        nc.sync.dma_start(id_sb[:], identIn)
        nc.sync.dma_start(ones_sb[:], onesIn)
        nc.sync.dma_start(wo_sb[:], woT.rearrange("(ko p) m -> p ko m", p=P))

        def rope(dst, src, n, tag):
            """dst = src*cos + rotate_half(src)*sin for token block n.

            dst: [P, TQ] bf16 AP; src: [P, TQ] fp32 AP (PSUM ok).
            ssin table is pre-negated on its top half."""
            c = cos_sb[:, n * TQ:(n + 1) * TQ]
            s = sin_sb[:, n * TQ:(n + 1) * TQ]
            rt = stage.tile([P, TQ], f32, tag="rt", name=f"rt_{n}_{tag}")
            t2 = stage.tile([P, TQ], f32, tag="rt2", name=f"r2_{n}_{tag}")
            nc.vector.tensor_copy(rt[0:64, :], src[64:P, :])
            nc.vector.tensor_copy(rt[64:P, :], src[0:64, :])
            nc.vector.tensor_tensor(rt[:], rt[:], s, OP.mult)
            nc.vector.tensor_tensor(t2[:], src, c, OP.mult)
            nc.vector.tensor_tensor(dst, t2[:], rt[:], OP.add)

        # ---------- output-projection work feeder ----------
        # Chunk c's projection = 2 halves x (4 k-groups x 2 m-tiles x 8 k) of
        # bf16 matmuls + 4 evictions, emitted between attention/QKV steps of
        # block c+2 so the PE queue always has independent, *ready* work
        # (the chunk's gather landed a full block earlier). at-loads ride
        # the vector queue: their data is always ready, so they never
        # head-of-line-block it (sync carries the x prefetches, which wait).
        def feeder_load(c):
            units = []
            for half in range(2):
                for g in range(4):
                    units.append(("dma", half, g))
                    for m in (half * 2, half * 2 + 1):
                        for k8 in range(8):
                            units.append(("mm", half, g, m, k8))
                units.append(("evict", half * 2))
                units.append(("evict", half * 2 + 1))
            return {"c": c, "units": units, "pos": 0, "at": {}, "ps": {}}

        def feeder_load_tail(c):
            # per-head-gather variant: k visits head-major (k = c'*4 + h) so
            # each section only needs gather h; PSUM accumulation order-free
            units = []
            for half in range(2):
                for h in range(QH):
                    units.append(("tdma", half, h))
                    for m in (half * 2, half * 2 + 1):
                        for cc in range(NCORES):
                            units.append(("tmm", half, h, m, cc))
                units.append(("evict", half * 2))
                units.append(("evict", half * 2 + 1))
            return {"c": c, "units": units, "pos": 0, "at": {}, "ps": {}}

        def feeder_emit(fds, nu):
            st = None
            for cand in fds:
                if cand is not None and cand["pos"] < len(cand["units"]):
                    st = cand
                    break
            if st is None:
                return
            c, units = st["c"], st["units"]
            for u in units[st["pos"]:st["pos"] + nu]:
                kind = u[0]
                if kind == "dma":
                    _, half, g = u
                    t = atpool.tile([P, 8, TQ], bf16, tag="at",
                                    name=f"at{c}_{half}_{g}")
                    nc.sync.dma_start(
                        t[:],
                        ag_out[c].rearrange("(ko p) t -> p ko t", p=P)[
                            :, g * 8:(g + 1) * 8, :])
                    st["at"][(half, g)] = t
                elif kind == "mm":
                    _, half, g, m, k8 = u
                    if g == 0 and k8 == 0:
                        st["ps"][m] = pop.tile([P, TQ], f32, tag=f"op{m % 2}",
                                               name=f"op{c}_{m}")
                    nc.tensor.matmul(
                        st["ps"][m][:], wo_sb[:, g * 8 + k8, m * P:(m + 1) * P],
                        st["at"][(half, g)][:, k8, :],
                        start=(g == 0 and k8 == 0), stop=(g == 3 and k8 == 7))
                elif kind == "tdma":
                    _, half, h = u
                    t = atpool.tile([P, 8, TQ], bf16, tag="at",
                                    name=f"att{half}_{h}")
                    nc.sync.dma_start(
                        t[:], agt_out[h].rearrange("(cp p) t -> p cp t", p=P))
                    st["at"][(half, h)] = t
                elif kind == "tmm":
                    _, half, h, m, cc = u
                    if h == 0 and cc == 0:
                        st["ps"][m] = pop.tile([P, TQ], f32, tag=f"op{m % 2}",
                                               name=f"opt_{m}")
                    nc.tensor.matmul(
                        st["ps"][m][:],
                        wo_sb[:, cc * QH + h, m * P:(m + 1) * P],
                        st["at"][(half, h)][:, cc, :],
                        start=(h == 0 and cc == 0),
                        stop=(h == QH - 1 and cc == NCORES - 1))
                elif kind == "pre":
                    u[1]()
                else:  # evict
                    _, m = u
                    ob = obpool.tile([P, TQ], f32, tag="ob", name=f"ob{c}_{m}")
                    nc.scalar.copy(ob[:], st["ps"][m][:])
                    nc.gpsimd.dma_start(
                        outT[m * P:(m + 1) * P, c * TQ:(c + 1) * TQ], ob[:])
            st["pos"] = min(st["pos"] + nu, len(units))

        def feeder_flush(fds):
            for st in fds:
                if st is not None:
                    feeder_emit([st], len(st["units"]))

        # ---------- main merged loop ----------
        feeders = [feeder_load(c) for c in range(NCHUNK - 1)]
        feeders.append(feeder_load_tail(NCHUNK - 1))
        xq = x0_tiles
        pre_done = {}        # ("q", j) -> prefilled PSUM tile for this block
        for b in range(B):
            kT_cache = kvpool.tile([P, S], bf16, tag="kT")
            v_cache = kvpool.tile([P, S // P, HEAD_DIM], bf16, tag="v")
            for n in range(NB):
                ch = b * NB + n
                # block ch drains chunk ch-2 (its gather landed a block ago);
                # the last block additionally drains chunk 6.
                fds = [feeders[ch - 2]] if ch >= 2 else []
                if ch == NCHUNK - 1:
                    fds.append(feeders[ch - 1])

                # ---- QKV: six chains over the full contraction ----
                qT_sb = qpool.tile([P, QH, TQ], bf16, tag="q")
                chains = (
                    [(wq_sb[j], ("q", j)) for j in range(QH)]
                    + [(wk_sb, ("k",)), (wv_sb, ("v",))]
                )
                for ci, (w_t, what) in enumerate(chains):
                    if what in pre_done:
                        ps = pre_done.pop(what)
                    else:
                        ps = pq.tile([P, TQ], f32, tag="qkv",
                                     name=f"qkv{ch}_{what}")
                        for k in range(KT):
                            nc.tensor.matmul(
                                ps[:], w_t[:, k, :], xq[k // 8][:, k % 8, :],
                                start=(k == 0), stop=(k == KT - 1))
                    feeder_emit(fds, 1 if ci == 0 else (4 if ci >= 2 else 0))
                    if what[0] == "q":
                        rope(qT_sb[:, what[1], :], ps[:], n, f"q{what[1]}")
                    elif what[0] == "k":
                        rope(kT_cache[:, n * TQ:(n + 1) * TQ], ps[:], n, "k")
                    else:
                        vT_sb = stage.tile([P, TQ], f32, tag="vt",
                                           name=f"vt{ch}")
                        nc.scalar.copy(vT_sb[:], ps[:])
                        for i in range(4):
                            tp = pst.tile([P, TQ], f32, tag="st",
                                          name=f"vtr{ch}_{i}")
                            nc.tensor.transpose(
                                tp[:, :P], vT_sb[:, i * P:(i + 1) * P],
                                id_sb[:])
                            nc.vector.tensor_copy(
                                v_cache[:, n * 4 + i, :], tp[:, :P])

                # lowest-priority filler: the next block's first two QKV
                # chains, prefilled into spare pq banks during this block's
                # attention (always-ready work for when the feeders run dry)
                xq_next, pre_next = xq, {}
                if ch < NCHUNK - 1:
                    b2, n2 = divmod(ch + 1, NB)
                    xq_next = issue_x(b2, n2)

                    def pre_unit(j, k, xq_n=xq_next, pp=pre_next, chn=ch + 1):
                        def fn():
                            if k == 0:
                                pp[("q", j)] = pq.tile(
                                    [P, TQ], f32, tag="qkv",
                                    name=f"pre{chn}_{j}")
                            nc.tensor.matmul(
                                pp[("q", j)][:], wq_sb[j][:, k, :],
                                xq_n[k // 8][:, k % 8, :],
                                start=(k == 0), stop=(k == KT - 1))
                        return fn

                    fds.append({
                        "c": -1, "pos": 0, "at": {}, "ps": {},
                        "units": [("pre", pre_unit(j, k))
                                  for j in (0, 1) for k in range(KT)],
                    })

                # ---- attention; pv(t-1) issued after scores(t) so the PE
                # never waits on the exp of the tile it just produced.
                # The last block keeps its in-attention drain rate low so
                # enough ready projection work remains to cover the final
                # chunk's gather latency after attention ends. ----
                rate = 1 if ch == NCHUNK - 1 else 2
                ntk = (n + 1) * (TQ // P)
                for h in range(QH):
                    acc = aux.tile([P, TQ], f32r, tag="acc", name=f"ac{ch}{h}")
                    pv_ps = ppv.tile([P, TQ], f32, tag="pv", name=f"pv{ch}{h}")
                    pend = None  # (es, c0, t) awaiting its pv matmul
                    for t in range(ntk):
                        dg = t - (ntk - 4)  # diagonal tile index, >=0 on diag
                        c0 = dg * P if dg > 0 else 0
                        st_ps = pst.tile([P, TQ], f32, tag="st",
                                         name=f"st{ch}_{h}_{t}")
                        nc.tensor.matmul(
                            st_ps[:, c0:], kT_cache[:, t * P:(t + 1) * P],
                            qT_sb[:, h, c0:], start=True, stop=True)
                        if pend is not None:
                            pes, pc0, pt = pend
                            nc.tensor.matmul(
                                pv_ps[:, pc0:], v_cache[:, pt, :],
                                pes[:, pc0:], start=(pt == 0), stop=False,
                                skip_group_check=True)
                        es = epool.tile([P, TQ], bf16, tag="es",
                                        name=f"es{ch}_{h}_{t}")
                        nc.scalar.activation(es[:, c0:], st_ps[:, c0:], AF.Exp)
                        if dg >= 0:
                            nc.vector.tensor_tensor(
                                es[:, c0:], es[:, c0:], mask_sb[:, dg, c0:],
                                OP.mult)
                        if t == 0:
                            nc.vector.tensor_copy(acc[:], es[:])
                        else:
                            nc.vector.tensor_tensor(
                                acc[:, c0:], acc[:, c0:].bitcast(f32),
                                es[:, c0:], OP.add)
                        pend = (es, c0, t)
                        feeder_emit(fds, rate)
                    pes, pc0, pt = pend
                    nc.tensor.matmul(
                        pv_ps[:, pc0:], v_cache[:, pt, :], pes[:, pc0:],
                        start=(pt == 0), stop=True, skip_group_check=True)
                    # denominator + normalize
                    dn_ps = pst.tile([P, TQ], f32, tag="st", name=f"dn{ch}{h}")
                    nc.tensor.matmul(dn_ps[:1, :], ones_sb[:], acc[:],
                                     start=True, stop=True)
                    rec = aux.tile([1, TQ], f32, tag="rec", name=f"rc{ch}{h}")
                    nc.vector.reciprocal_approx_fast(rec[:], dn_ps[:1, :])
                    bc = aux.tile([P, TQ], f32, tag="bc", name=f"bc{ch}{h}")
                    nc.gpsimd.partition_broadcast(bc[:], rec[:])
                    ao = aux.tile([P, TQ], bf16, tag="ao", name=f"ao{ch}{h}")
                    nc.vector.tensor_tensor(ao[:], pv_ps[:], bc[:], OP.mult)
                    if ch < NCHUNK - 1:
                        nc.gpsimd.dma_start(
                            ag_in[ch][h * P:(h + 1) * P, :], ao[:])
                    else:
                        nc.gpsimd.dma_start(agt_in[h][:, :], ao[:])
                        if not getattr(nc, "_skip_collectives", False):
                            nc.gpsimd.collective_compute(
                                "AllGather",
                                mybir.AluOpType.bypass,
                                replica_groups=[list(range(NCORES))],
                                ins=[agt_in[h].opt()],
                                outs=[agt_out[h].opt()],
                            )
                    feeder_emit(fds, 2)

                # ---- AllGather this chunk across the 8 cores ----
                if ch < NCHUNK - 1 and not getattr(
                        nc, "_skip_collectives", False):
                    nc.gpsimd.collective_compute(
                        "AllGather",
                        mybir.AluOpType.bypass,
                        replica_groups=[list(range(NCORES))],
                        ins=[ag_in[ch].opt()],
                        outs=[ag_out[ch].opt()],
                    )
                feeder_flush(fds)
                xq, pre_done = xq_next, pre_next

        # tail: the last chunk's remaining projection work
        feeder_flush([feeders[NCHUNK - 1]])


_NC_CACHE = None


def _get_module():
    global _NC_CACHE
    if _NC_CACHE is None:
        _NC_CACHE = _build_module()
    return _NC_CACHE


def _host_consts():
    inv_freq = 1.0 / (ROPE_THETA ** (np.arange(0, HEAD_DIM, 2,
                                               dtype=np.float32) / HEAD_DIM))
    t = np.arange(S, dtype=np.float32)
    freqs = np.outer(t, inv_freq).astype(np.float32)      # [S, 64]
    cos_h = np.cos(freqs).T                               # [64, S]
    sin_h = np.sin(freqs).T
    cosT = np.ascontiguousarray(
        np.concatenate([cos_h, cos_h], axis=0)).astype(BF)
    ssinT = np.ascontiguousarray(
        np.concatenate([-sin_h, sin_h], axis=0)).astype(BF)

    i = np.arange(P)[:, None]
    j = np.arange(TQ)[None, :]
    masks = np.concatenate(
        [(i + r * P <= j).astype(np.float32) for r in range(4)], axis=1
    ).astype(BF)                                          # [128, 4*512]
    ident = np.eye(P, dtype=np.float32)
    ones = np.ones((P, 1), dtype=np.float32)
    return cosT, ssinT, masks, ident, ones


def make_in_maps(hidden_states, wq, wk, wv, wo):
    hidden_states = np.asarray(hidden_states, dtype=np.float32)
    wq = np.asarray(wq, dtype=np.float32)
    wk = np.asarray(wk, dtype=np.float32)
    wv = np.asarray(wv, dtype=np.float32)
    wo = np.asarray(wo, dtype=np.float32)

    xT = np.ascontiguousarray(
        hidden_states.reshape(TOK, HIDDEN).T).astype(BF)
    cosT, ssinT, masks, ident, ones = _host_consts()
    qscale = 1.0 / math.sqrt(HEAD_DIM)

    in_maps = []
    for c in range(NCORES):
        in_maps.append({
            "xT": xT,
            "wqT": np.ascontiguousarray(
                (wq[c * HG:(c + 1) * HG] * qscale).T).astype(BF),
            "wkT": np.ascontiguousarray(
                wk[c * HEAD_DIM:(c + 1) * HEAD_DIM].T).astype(BF),
            "wvT": np.ascontiguousarray(
                wv[c * HEAD_DIM:(c + 1) * HEAD_DIM].T).astype(BF),
            "woT": np.ascontiguousarray(wo[c * HG:(c + 1) * HG].T).astype(BF),
            "cosIn": cosT,
            "ssinIn": ssinT,
            "masksIn": masks,
            "identIn": ident,
            "onesIn": ones,
        })
    return in_maps


def assemble_output(results):
    out = np.empty((TOK, HIDDEN), dtype=np.float32)
    for c in range(NCORES):
        out[:, c * HG:(c + 1) * HG] = results[c]["outT"].T
    return out.reshape(B, S, HIDDEN)


def kernel(hidden_states, wq, wk, wv, wo):
    nc = _get_module()
    in_maps = make_in_maps(hidden_states, wq, wk, wv, wo)
    trace = bool(int(os.environ.get("KERNEL_TRACE", "0")))
    res = bass_utils.run_bass_kernel_spmd(
        nc, in_maps, core_ids=list(range(NCORES)), trace=trace
    )
    if trace:
        kernel.last_results = res
    return assemble_output(res.results)


kernel.last_results = None


# revision 64
# speedup vs baseline: 1.0530x; 1.0530x over previous
"""InternLM3 attention block on 8 Trainium2 NeuronCores (Bass/Tile), v2.

Sharding (tensor-parallel over heads, using the GQA structure):
  core c owns Q heads [4c,4c+4) and KV head c; per-core fused pipeline over
  512-token blocks; attention outputs AllGathered in 8 chunks; each core
  computes its 512-column slice of the output projection.

v2 performance structure (vs v1):
  - QKV + output-projection GEMMs run in bf16 (halved LDWEIGHTS cost and
    SBUF footprint; fp32 PSUM accumulation keeps error ~1e-3).
  - Single merged loop: the output projection of chunk ch-1 is interleaved
    into the attention of chunk ch via a work feeder, so the PE array has
    independent GEMM work to fill softmax dependency bubbles and there is
    no serial projection tail.
  - QKV as six per-output chains over the full contraction (2 PSUM banks
    instead of 6), leaving banks for scores(2)/pv(1)/outproj(2).
  - Causal diagonal tiles restrict matmul/exp/mask work to the unmasked
    query range (free-dim subrange), cutting ~30% of attention rows.
  - RoPE reads Q/K straight from PSUM with partition-shifted multiplies
    (no staging copies); v transposed via PE identity matmul.
  - Denominator: DVE accumulation + ones-matmul, reciprocal_approx_fast,
    gpsimd partition_broadcast.
  - DMA queues split: x/weights/at on sync, ao/out stores on gpsimd.
"""

import math
import os
import sys

if "/opt/trn_rl_repo" not in sys.path:
    sys.path.insert(0, "/opt/trn_rl_repo")

import numpy as np
import ml_dtypes

import concourse.bass as bass
import concourse.mybir as mybir
import concourse.tile as tile
from concourse import bacc
from concourse import bass_utils

# ---- problem constants (hardcoded per harness contract) ----
HIDDEN = 4096
N_HEADS = 32
N_KV_HEADS = 8
HEAD_DIM = 128
ROPE_THETA = 10000.0
B, S = 2, 2048
NCORES = 8

P = 128
TQ = 512                      # token block
NB = S // TQ                  # 4 blocks per batch
KT = HIDDEN // P              # 32 contraction tiles
QH = N_HEADS // NCORES        # 4 q-heads per core
HG = QH * HEAD_DIM            # 512 = head-group width per core
NCHUNK = B * NB               # 8 allgather chunks
TOK = B * S                   # 4096 tokens

f32 = mybir.dt.float32
f32r = mybir.dt.float32r
bf16 = mybir.dt.bfloat16
BF = ml_dtypes.bfloat16


def _build_module(with_collectives=True):
    nc = bacc.Bacc("TRN2", target_bir_lowering=False, debug=False,
                   num_devices=NCORES)
    nc._skip_collectives = not with_collectives

    xT = nc.dram_tensor("xT", [HIDDEN, TOK], bf16, kind="ExternalInput").ap()
    wqT = nc.dram_tensor("wqT", [HIDDEN, HG], bf16, kind="ExternalInput").ap()
    wkT = nc.dram_tensor("wkT", [HIDDEN, HEAD_DIM], bf16,
                         kind="ExternalInput").ap()
    wvT = nc.dram_tensor("wvT", [HIDDEN, HEAD_DIM], bf16,
                         kind="ExternalInput").ap()
    woT = nc.dram_tensor("woT", [HIDDEN, HG], bf16, kind="ExternalInput").ap()
    cosIn = nc.dram_tensor("cosIn", [P, S], bf16, kind="ExternalInput").ap()
    ssinIn = nc.dram_tensor("ssinIn", [P, S], bf16, kind="ExternalInput").ap()
    masksIn = nc.dram_tensor("masksIn", [P, 4 * TQ], bf16,
                             kind="ExternalInput").ap()
    identIn = nc.dram_tensor("identIn", [P, P], f32, kind="ExternalInput").ap()
    onesIn = nc.dram_tensor("onesIn", [P, 1], f32r, kind="ExternalInput").ap()
    outT = nc.dram_tensor("outT", [HG, TOK], f32, kind="ExternalOutput").ap()

    ag_in = [
        nc.dram_tensor(f"ag_in{i}", [HG, TQ], bf16, kind="Internal").ap()
        for i in range(NCHUNK - 1)
    ]
    ag_out = [
        nc.dram_tensor(f"ag_out{i}", [HIDDEN, TQ], bf16, kind="Internal",
                       addr_space="Shared").ap()
        for i in range(NCHUNK - 1)
    ]
    # last chunk gathers per head so the final projection can start before
    # the whole block's attention (and its laggard cores) finish
    agt_in = [
        nc.dram_tensor(f"agt_in{h}", [P, TQ], bf16, kind="Internal").ap()
        for h in range(QH)
    ]
    agt_out = [
        nc.dram_tensor(f"agt_out{h}", [NCORES * P, TQ], bf16,
                       kind="Internal", addr_space="Shared").ap()
        for h in range(QH)
    ]

    with tile.TileContext(nc) as tc:
        _body(tc, nc, xT, wqT, wkT, wvT, woT, cosIn, ssinIn, masksIn, identIn,
              onesIn, outT, ag_in, ag_out, agt_in, agt_out)
    nc.compile()
    return nc


def _body(tc, nc, xT, wqT, wkT, wvT, woT, cosIn, ssinIn, masksIn, identIn,
          onesIn, outT, ag_in, ag_out, agt_in, agt_out):
    AF = mybir.ActivationFunctionType
    OP = mybir.AluOpType

    with (
        tc.tile_pool(name="wpool", bufs=1) as wpool,
        tc.tile_pool(name="xpool", bufs=4) as xpool,
        tc.tile_pool(name="kvpool", bufs=1) as kvpool,
        tc.tile_pool(name="qpool", bufs=1) as qpool,
        tc.tile_pool(name="stage", bufs=1) as stage,
        tc.tile_pool(name="epool", bufs=2) as epool,
        tc.tile_pool(name="aux", bufs=2) as aux,
        tc.tile_pool(name="atpool", bufs=3) as atpool,
        tc.tile_pool(name="obpool", bufs=2) as obpool,
        tc.tile_pool(name="pq", bufs=2, space="PSUM") as pq,
        tc.tile_pool(name="ppv", bufs=1, space="PSUM") as ppv,
        tc.tile_pool(name="pst", bufs=3, space="PSUM") as pst,
        tc.tile_pool(name="pop", bufs=1, space="PSUM") as pop,
    ):
        # ---- resident weight/const tiles (DMAs issued interleaved below) --
        wq_sb = [wpool.tile([P, KT, P], bf16, tag=f"wq{j}", name=f"wq{j}")
                 for j in range(QH)]
        wk_sb = wpool.tile([P, KT, P], bf16, tag="wk")
        wv_sb = wpool.tile([P, KT, P], bf16, tag="wv")
        wo_sb = wpool.tile([P, KT, HG], bf16, tag="wo")
        cos_sb = wpool.tile([P, S], bf16, tag="cos")
        sin_sb = wpool.tile([P, S], bf16, tag="sin")
        mask_sb = wpool.tile([P, 4, TQ], bf16, tag="mask")
        id_sb = wpool.tile([P, P], f32, tag="ident")
        ones_sb = wpool.tile([P, 1], f32r, tag="ones")

        def w_dma(t, src, j=None):
            if j is None:
                nc.sync.dma_start(t[:], src)
            else:
                nc.sync.dma_start(
                    t[:], src[:, j * P:(j + 1) * P].rearrange(
                        "(ko p) m -> p ko m", p=P))

        def issue_x(b, n):
            """Issue the 4 quarter DMAs of x for token block (b, n)."""
            tok0 = b * S + n * TQ
            tiles = []
            for qt in range(4):
                t = xpool.tile([P, 8, TQ], bf16, tag="xq",
                               name=f"xq{b}_{n}_{qt}")
                nc.sync.dma_start(
                    t[:],
                    xT[qt * 8 * P:(qt + 1) * 8 * P, tok0:tok0 + TQ].rearrange(
                        "(ko p) t -> p ko t", p=P))
                tiles.append(t)
            return tiles

        # startup order: wq0, x(0,0) quarters interleaved with wq1-3, then
        # the rest. Keeps the first QKV chain start at ~6us.
        w_dma(wq_sb[0], wqT, 0)
        x0_tiles = issue_x(0, 0)
        for j in range(1, QH):
            w_dma(wq_sb[j], wqT, j)
        w_dma(wk_sb, wkT, 0)
        w_dma(wv_sb, wvT, 0)
        nc.sync.dma_start(cos_sb[:], cosIn)
        nc.sync.dma_start(sin_sb[:], ssinIn)
        nc.sync.dma_start(mask_sb[:], masksIn.rearrange("p (r t) -> p r t", r=4))
        nc.sync.dma_start(id_sb[:], identIn)
        nc.sync.dma_start(ones_sb[:], onesIn)
        nc.sync.dma_start(wo_sb[:], woT.rearrange("(ko p) m -> p ko m", p=P))

        def rope(dst, src, n, tag):
            """dst = src*cos + rotate_half(src)*sin for token block n.

            dst: [P, TQ] bf16 AP; src: [P, TQ] fp32 AP (PSUM ok).
            ssin table is pre-negated on its top half."""
            c = cos_sb[:, n * TQ:(n + 1) * TQ]
            s = sin_sb[:, n * TQ:(n + 1) * TQ]
            rt = stage.tile([P, TQ], f32, tag="rt", name=f"rt_{n}_{tag}")
            t2 = stage.tile([P, TQ], f32, tag="rt2", name=f"r2_{n}_{tag}")
            nc.vector.tensor_copy(rt[0:64, :], src[64:P, :])
            nc.vector.tensor_copy(rt[64:P, :], src[0:64, :])
            nc.vector.tensor_tensor(rt[:], rt[:], s, OP.mult)
            nc.vector.tensor_tensor(t2[:], src, c, OP.mult)
            nc.vector.tensor_tensor(dst, t2[:], rt[:], OP.add)

        # ---------- output-projection work feeder ----------
        # Chunk c's projection = 2 halves x (4 k-groups x 2 m-tiles x 8 k) of
        # bf16 matmuls + 4 evictions, emitted between attention/QKV steps of
        # block c+2 so the PE queue always has independent, *ready* work
        # (the chunk's gather landed a full block earlier). at-loads ride
        # the vector queue: their data is always ready, so they never
        # head-of-line-block it (sync carries the x prefetches, which wait).
        def feeder_load(c):
            units = []
            for half in range(2):
                for g in range(4):
                    units.append(("dma", half, g))
                    for m in (half * 2, half * 2 + 1):
                        for k8 in range(8):
                            units.append(("mm", half, g, m, k8))
                units.append(("evict", half * 2))
                units.append(("evict", half * 2 + 1))
            return {"c": c, "units": units, "pos": 0, "at": {}, "ps": {}}

        def feeder_load_tail(c):
            # per-head-gather variant: k visits head-major (k = c'*4 + h) so
            # each section only needs gather h; PSUM accumulation order-free
            units = []
            for half in range(2):
                for h in range(QH):
                    units.append(("tdma", half, h))
                    for m in (half * 2, half * 2 + 1):
                        for cc in range(NCORES):
                            units.append(("tmm", half, h, m, cc))
                units.append(("evict", half * 2))
                units.append(("evict", half * 2 + 1))
            return {"c": c, "units": units, "pos": 0, "at": {}, "ps": {}}

        def feeder_emit(fds, nu):
            st = None
            for cand in fds:
                if cand is not None and cand["pos"] < len(cand["units"]):
                    st = cand
                    break
            if st is None:
                return
            c, units = st["c"], st["units"]
            for u in units[st["pos"]:st["pos"] + nu]:
                kind = u[0]
                if kind == "dma":
                    _, half, g = u
                    t = atpool.tile([P, 8, TQ], bf16, tag="at",
                                    name=f"at{c}_{half}_{g}")
                    nc.sync.dma_start(
                        t[:],
                        ag_out[c].rearrange("(ko p) t -> p ko t", p=P)[
                            :, g * 8:(g + 1) * 8, :])
                    st["at"][(half, g)] = t
                elif kind == "mm":
                    _, half, g, m, k8 = u
                    if g == 0 and k8 == 0:
                        st["ps"][m] = pop.tile([P, TQ], f32, tag=f"op{m % 2}",
                                               name=f"op{c}_{m}")
                    nc.tensor.matmul(
                        st["ps"][m][:], wo_sb[:, g * 8 + k8, m * P:(m + 1) * P],
                        st["at"][(half, g)][:, k8, :],
                        start=(g == 0 and k8 == 0), stop=(g == 3 and k8 == 7))
                elif kind == "tdma":
                    _, half, h = u
                    t = atpool.tile([P, 8, TQ], bf16, tag="at",
                                    name=f"att{half}_{h}")
                    nc.sync.dma_start(
                        t[:], agt_out[h].rearrange("(cp p) t -> p cp t", p=P))
                    st["at"][(half, h)] = t
                elif kind == "tmm":
                    _, half, h, m, cc = u
                    if h == 0 and cc == 0:
                        st["ps"][m] = pop.tile([P, TQ], f32, tag=f"op{m % 2}",
                                               name=f"opt_{m}")
                    nc.tensor.matmul(
                        st["ps"][m][:],
                        wo_sb[:, cc * QH + h, m * P:(m + 1) * P],
                        st["at"][(half, h)][:, cc, :],
                        start=(h == 0 and cc == 0),
                        stop=(h == QH - 1 and cc == NCORES - 1))
                elif kind == "pre":
                    u[1]()
                else:  # evict
                    _, m = u
                    ob = obpool.tile([P, TQ], f32, tag="ob", name=f"ob{c}_{m}")
                    nc.scalar.copy(ob[:], st["ps"][m][:])
                    nc.gpsimd.dma_start(
                        outT[m * P:(m + 1) * P, c * TQ:(c + 1) * TQ], ob[:])
            st["pos"] = min(st["pos"] + nu, len(units))

        def feeder_flush(fds):
            for st in fds:
                if st is not None:
                    feeder_emit([st], len(st["units"]))

        # ---------- main merged loop ----------
        feeders = [feeder_load(c) for c in range(NCHUNK - 1)]
        feeders.append(feeder_load_tail(NCHUNK - 1))
        for b in range(B):
            kT_cache = kvpool.tile([P, S], bf16, tag="kT")
            v_cache = kvpool.tile([P, S // P, HEAD_DIM], bf16, tag="v")
            for n in range(NB):
                ch = b * NB + n
                xq = x0_tiles if ch == 0 else issue_x(b, n)
                # block ch drains chunk ch-2 (its gather landed a block ago);
                # the last block additionally drains chunk 6.
                fds = [feeders[ch - 2]] if ch >= 2 else []
                if ch == NCHUNK - 1:
                    fds.append(feeders[ch - 1])

                # ---- QKV: six chains over the full contraction ----
                qT_sb = qpool.tile([P, QH, TQ], bf16, tag="q")
                chains = (
                    [(wq_sb[j], ("q", j)) for j in range(QH)]
                    + [(wk_sb, ("k",)), (wv_sb, ("v",))]
                )
                for ci, (w_t, what) in enumerate(chains):
                    ps = pq.tile([P, TQ], f32, tag="qkv",
                                 name=f"qkv{ch}_{what}")
                    for k in range(KT):
                        nc.tensor.matmul(
                            ps[:], w_t[:, k, :], xq[k // 8][:, k % 8, :],
                            start=(k == 0), stop=(k == KT - 1))
                    feeder_emit(fds, 1 if ci == 0 else (4 if ci >= 2 else 0))
                    if what[0] == "q":
                        rope(qT_sb[:, what[1], :], ps[:], n, f"q{what[1]}")
                    elif what[0] == "k":
                        rope(kT_cache[:, n * TQ:(n + 1) * TQ], ps[:], n, "k")
                    else:
                        vT_sb = stage.tile([P, TQ], f32, tag="vt",
                                           name=f"vt{ch}")
                        nc.scalar.copy(vT_sb[:], ps[:])
                        for i in range(4):
                            tp = pst.tile([P, TQ], f32, tag="st",
                                          name=f"vtr{ch}_{i}")
                            nc.tensor.transpose(
                                tp[:, :P], vT_sb[:, i * P:(i + 1) * P],
                                id_sb[:])
                            nc.vector.tensor_copy(
                                v_cache[:, n * 4 + i, :], tp[:, :P])

                # ---- attention; pv(t-1) issued after scores(t) so the PE
                # never waits on the exp of the tile it just produced.
                # The last block keeps its in-attention drain rate low so
                # enough ready projection work remains to cover the final
                # chunk's gather latency after attention ends. ----
                rate = 1 if ch == NCHUNK - 1 else 2
                ntk = (n + 1) * (TQ // P)
                for h in range(QH):
                    acc = aux.tile([P, TQ], f32r, tag="acc", name=f"ac{ch}{h}")
                    pv_ps = ppv.tile([P, TQ], f32, tag="pv", name=f"pv{ch}{h}")
                    pend = None  # (es, c0, t) awaiting its pv matmul
                    for t in range(ntk):
                        dg = t - (ntk - 4)  # diagonal tile index, >=0 on diag
                        c0 = dg * P if dg > 0 else 0
                        st_ps = pst.tile([P, TQ], f32, tag="st",
                                         name=f"st{ch}_{h}_{t}")
                        nc.tensor.matmul(
                            st_ps[:, c0:], kT_cache[:, t * P:(t + 1) * P],
                            qT_sb[:, h, c0:], start=True, stop=True)
                        if pend is not None:
                            pes, pc0, pt = pend
                            nc.tensor.matmul(
                                pv_ps[:, pc0:], v_cache[:, pt, :],
                                pes[:, pc0:], start=(pt == 0), stop=False,
                                skip_group_check=True)
                        es = epool.tile([P, TQ], bf16, tag="es",
                                        name=f"es{ch}_{h}_{t}")
                        nc.scalar.activation(es[:, c0:], st_ps[:, c0:], AF.Exp)
                        if dg >= 0:
                            nc.vector.tensor_tensor(
                                es[:, c0:], es[:, c0:], mask_sb[:, dg, c0:],
                                OP.mult)
                        if t == 0:
                            nc.vector.tensor_copy(acc[:], es[:])
                        else:
                            nc.vector.tensor_tensor(
                                acc[:, c0:], acc[:, c0:].bitcast(f32),
                                es[:, c0:], OP.add)
                        pend = (es, c0, t)
                        feeder_emit(fds, rate)
                    pes, pc0, pt = pend
                    nc.tensor.matmul(
                        pv_ps[:, pc0:], v_cache[:, pt, :], pes[:, pc0:],
                        start=(pt == 0), stop=True, skip_group_check=True)
                    # denominator + normalize
                    dn_ps = pst.tile([P, TQ], f32, tag="st", name=f"dn{ch}{h}")
                    nc.tensor.matmul(dn_ps[:1, :], ones_sb[:], acc[:],
                                     start=True, stop=True)
                    rec = aux.tile([1, TQ], f32, tag="rec", name=f"rc{ch}{h}")
                    nc.vector.reciprocal_approx_fast(rec[:], dn_ps[:1, :])
                    bc = aux.tile([P, TQ], f32, tag="bc", name=f"bc{ch}{h}")
                    nc.gpsimd.partition_broadcast(bc[:], rec[:])
                    ao = aux.tile([P, TQ], bf16, tag="ao", name=f"ao{ch}{h}")
                    nc.vector.tensor_tensor(ao[:], pv_ps[:], bc[:], OP.mult)
                    if ch < NCHUNK - 1:
                        nc.gpsimd.dma_start(
                            ag_in[ch][h * P:(h + 1) * P, :], ao[:])
                    else:
                        nc.gpsimd.dma_start(agt_in[h][:, :], ao[:])
                        if not getattr(nc, "_skip_collectives", False):
                            nc.gpsimd.collective_compute(
                                "AllGather",
                                mybir.AluOpType.bypass,
                                replica_groups=[list(range(NCORES))],
                                ins=[agt_in[h].opt()],
                                outs=[agt_out[h].opt()],
                            )
                    feeder_emit(fds, 2)

                # ---- AllGather this chunk across the 8 cores ----
                if ch < NCHUNK - 1 and not getattr(
                        nc, "_skip_collectives", False):
                    nc.gpsimd.collective_compute(
                        "AllGather",
                        mybir.AluOpType.bypass,
                        replica_groups=[list(range(NCORES))],
                        ins=[ag_in[ch].opt()],
                        outs=[ag_out[ch].opt()],
                    )
                feeder_flush(fds)

        # tail: the last chunk's remaining projection work
        feeder_flush([feeders[NCHUNK - 1]])


_NC_CACHE = None


def _get_module():
    global _NC_CACHE
    if _NC_CACHE is None:
        _NC_CACHE = _build_module()
    return _NC_CACHE


def _host_consts():
    inv_freq = 1.0 / (ROPE_THETA ** (np.arange(0, HEAD_DIM, 2,
                                               dtype=np.float32) / HEAD_DIM))
    t = np.arange(S, dtype=np.float32)
    freqs = np.outer(t, inv_freq).astype(np.float32)      # [S, 64]
    cos_h = np.cos(freqs).T                               # [64, S]
    sin_h = np.sin(freqs).T
    cosT = np.ascontiguousarray(
        np.concatenate([cos_h, cos_h], axis=0)).astype(BF)
    ssinT = np.ascontiguousarray(
        np.concatenate([-sin_h, sin_h], axis=0)).astype(BF)

    i = np.arange(P)[:, None]
    j = np.arange(TQ)[None, :]
    masks = np.concatenate(
        [(i + r * P <= j).astype(np.float32) for r in range(4)], axis=1
    ).astype(BF)                                          # [128, 4*512]
    ident = np.eye(P, dtype=np.float32)
    ones = np.ones((P, 1), dtype=np.float32)
    return cosT, ssinT, masks, ident, ones


def make_in_maps(hidden_states, wq, wk, wv, wo):
    hidden_states = np.asarray(hidden_states, dtype=np.float32)
    wq = np.asarray(wq, dtype=np.float32)
    wk = np.asarray(wk, dtype=np.float32)
    wv = np.asarray(wv, dtype=np.float32)
    wo = np.asarray(wo, dtype=np.float32)

    xT = np.ascontiguousarray(
        hidden_states.reshape(TOK, HIDDEN).T).astype(BF)
    cosT, ssinT, masks, ident, ones = _host_consts()
    qscale = 1.0 / math.sqrt(HEAD_DIM)

    in_maps = []
    for c in range(NCORES):
        in_maps.append({
            "xT": xT,
            "wqT": np.ascontiguousarray(
                (wq[c * HG:(c + 1) * HG] * qscale).T).astype(BF),
            "wkT": np.ascontiguousarray(
                wk[c * HEAD_DIM:(c + 1) * HEAD_DIM].T).astype(BF),
            "wvT": np.ascontiguousarray(
                wv[c * HEAD_DIM:(c + 1) * HEAD_DIM].T).astype(BF),
            "woT": np.ascontiguousarray(wo[c * HG:(c + 1) * HG].T).astype(BF),
            "cosIn": cosT,
            "ssinIn": ssinT,
            "masksIn": masks,
            "identIn": ident,
            "onesIn": ones,
        })
    return in_maps


def assemble_output(results):
    out = np.empty((TOK, HIDDEN), dtype=np.float32)
    for c in range(NCORES):
        out[:, c * HG:(c + 1) * HG] = results[c]["outT"].T
    return out.reshape(B, S, HIDDEN)


def kernel(hidden_states, wq, wk, wv, wo):
    nc = _get_module()
    in_maps = make_in_maps(hidden_states, wq, wk, wv, wo)
    trace = bool(int(os.environ.get("KERNEL_TRACE", "0")))
    res = bass_utils.run_bass_kernel_spmd(
        nc, in_maps, core_ids=list(range(NCORES)), trace=trace
    )
    if trace:
        kernel.last_results = res
    return assemble_output(res.results)


kernel.last_results = None


# revision 65
# speedup vs baseline: 1.0632x; 1.0097x over previous
"""InternLM3 attention block on 8 Trainium2 NeuronCores (Bass/Tile), v2.

Sharding (tensor-parallel over heads, using the GQA structure):
  core c owns Q heads [4c,4c+4) and KV head c; per-core fused pipeline over
  512-token blocks; attention outputs AllGathered in 8 chunks; each core
  computes its 512-column slice of the output projection.

v2 performance structure (vs v1):
  - QKV + output-projection GEMMs run in bf16 (halved LDWEIGHTS cost and
    SBUF footprint; fp32 PSUM accumulation keeps error ~1e-3).
  - Single merged loop: the output projection of chunk ch-1 is interleaved
    into the attention of chunk ch via a work feeder, so the PE array has
    independent GEMM work to fill softmax dependency bubbles and there is
    no serial projection tail.
  - QKV as six per-output chains over the full contraction (2 PSUM banks
    instead of 6), leaving banks for scores(2)/pv(1)/outproj(2).
  - Causal diagonal tiles restrict matmul/exp/mask work to the unmasked
    query range (free-dim subrange), cutting ~30% of attention rows.
  - RoPE reads Q/K straight from PSUM with partition-shifted multiplies
    (no staging copies); v transposed via PE identity matmul.
  - Denominator: DVE accumulation + ones-matmul, reciprocal_approx_fast,
    gpsimd partition_broadcast.
  - DMA queues split: x/weights/at on sync, ao/out stores on gpsimd.
"""

import math
import os
import sys

if "/opt/trn_rl_repo" not in sys.path:
    sys.path.insert(0, "/opt/trn_rl_repo")

import numpy as np
import ml_dtypes

import concourse.bass as bass
import concourse.mybir as mybir
import concourse.tile as tile
from concourse import bacc
from concourse import bass_utils

# ---- problem constants (hardcoded per harness contract) ----
HIDDEN = 4096
N_HEADS = 32
N_KV_HEADS = 8
HEAD_DIM = 128
ROPE_THETA = 10000.0
B, S = 2, 2048
NCORES = 8

P = 128
TQ = 512                      # token block
NB = S // TQ                  # 4 blocks per batch
KT = HIDDEN // P              # 32 contraction tiles
QH = N_HEADS // NCORES        # 4 q-heads per core
HG = QH * HEAD_DIM            # 512 = head-group width per core
NCHUNK = B * NB               # 8 allgather chunks
TOK = B * S                   # 4096 tokens

f32 = mybir.dt.float32
f32r = mybir.dt.float32r
bf16 = mybir.dt.bfloat16
BF = ml_dtypes.bfloat16


def _build_module(with_collectives=True):
    nc = bacc.Bacc("TRN2", target_bir_lowering=False, debug=False,
                   num_devices=NCORES)
    nc._skip_collectives = not with_collectives

    xT = nc.dram_tensor("xT", [HIDDEN, TOK], bf16, kind="ExternalInput").ap()
    wqT = nc.dram_tensor("wqT", [HIDDEN, HG], bf16, kind="ExternalInput").ap()
    wkT = nc.dram_tensor("wkT", [HIDDEN, HEAD_DIM], bf16,
                         kind="ExternalInput").ap()
    wvT = nc.dram_tensor("wvT", [HIDDEN, HEAD_DIM], bf16,
                         kind="ExternalInput").ap()
    woT = nc.dram_tensor("woT", [HIDDEN, HG], bf16, kind="ExternalInput").ap()
    cosIn = nc.dram_tensor("cosIn", [P, S], bf16, kind="ExternalInput").ap()
    ssinIn = nc.dram_tensor("ssinIn", [P, S], bf16, kind="ExternalInput").ap()
    masksIn = nc.dram_tensor("masksIn", [P, 4 * TQ], bf16,
                             kind="ExternalInput").ap()
    identIn = nc.dram_tensor("identIn", [P, P], f32, kind="ExternalInput").ap()
    onesIn = nc.dram_tensor("onesIn", [P, 1], f32r, kind="ExternalInput").ap()
    outT = nc.dram_tensor("outT", [HG, TOK], f32, kind="ExternalOutput").ap()

    ag_in = [
        nc.dram_tensor(f"ag_in{i}", [HG, TQ], bf16, kind="Internal").ap()
        for i in range(NCHUNK - 1)
    ]
    ag_out = [
        nc.dram_tensor(f"ag_out{i}", [HIDDEN, TQ], bf16, kind="Internal",
                       addr_space="Shared").ap()
        for i in range(NCHUNK - 1)
    ]
    # last chunk gathers per head so the final projection can start before
    # the whole block's attention (and its laggard cores) finish
    agt_in = [
        nc.dram_tensor(f"agt_in{h}", [P, TQ], bf16, kind="Internal").ap()
        for h in range(QH)
    ]
    agt_out = [
        nc.dram_tensor(f"agt_out{h}", [NCORES * P, TQ], bf16,
                       kind="Internal", addr_space="Shared").ap()
        for h in range(QH)
    ]

    with tile.TileContext(nc) as tc:
        _body(tc, nc, xT, wqT, wkT, wvT, woT, cosIn, ssinIn, masksIn, identIn,
              onesIn, outT, ag_in, ag_out, agt_in, agt_out)
    nc.compile()
    return nc


def _body(tc, nc, xT, wqT, wkT, wvT, woT, cosIn, ssinIn, masksIn, identIn,
          onesIn, outT, ag_in, ag_out, agt_in, agt_out):
    AF = mybir.ActivationFunctionType
    OP = mybir.AluOpType

    with (
        tc.tile_pool(name="wpool", bufs=1) as wpool,
        tc.tile_pool(name="xpool", bufs=4) as xpool,
        tc.tile_pool(name="kvpool", bufs=1) as kvpool,
        tc.tile_pool(name="qpool", bufs=1) as qpool,
        tc.tile_pool(name="stage", bufs=1) as stage,
        tc.tile_pool(name="epool", bufs=3) as epool,
        tc.tile_pool(name="aux", bufs=2) as aux,
        tc.tile_pool(name="atpool", bufs=3) as atpool,
        tc.tile_pool(name="obpool", bufs=2) as obpool,
        tc.tile_pool(name="pq", bufs=2, space="PSUM") as pq,
        tc.tile_pool(name="ppv", bufs=1, space="PSUM") as ppv,
        tc.tile_pool(name="pst", bufs=3, space="PSUM") as pst,
        tc.tile_pool(name="pop", bufs=1, space="PSUM") as pop,
    ):
        # ---- resident weight/const tiles (DMAs issued interleaved below) --
        wq_sb = [wpool.tile([P, KT, P], bf16, tag=f"wq{j}", name=f"wq{j}")
                 for j in range(QH)]
        wk_sb = wpool.tile([P, KT, P], bf16, tag="wk")
        wv_sb = wpool.tile([P, KT, P], bf16, tag="wv")
        wo_sb = wpool.tile([P, KT, HG], bf16, tag="wo")
        cos_sb = wpool.tile([P, S], bf16, tag="cos")
        sin_sb = wpool.tile([P, S], bf16, tag="sin")
        mask_sb = wpool.tile([P, 4, TQ], bf16, tag="mask")
        id_sb = wpool.tile([P, P], f32, tag="ident")
        ones_sb = wpool.tile([P, 1], f32r, tag="ones")

        def w_dma(t, src, j=None):
            if j is None:
                nc.sync.dma_start(t[:], src)
            else:
                nc.sync.dma_start(
                    t[:], src[:, j * P:(j + 1) * P].rearrange(
                        "(ko p) m -> p ko m", p=P))

        def issue_x(b, n):
            """Issue the 4 quarter DMAs of x for token block (b, n)."""
            tok0 = b * S + n * TQ
            tiles = []
            for qt in range(4):
                t = xpool.tile([P, 8, TQ], bf16, tag="xq",
                               name=f"xq{b}_{n}_{qt}")
                nc.sync.dma_start(
                    t[:],
                    xT[qt * 8 * P:(qt + 1) * 8 * P, tok0:tok0 + TQ].rearrange(
                        "(ko p) t -> p ko t", p=P))
                tiles.append(t)
            return tiles

        # startup order: wq0, x(0,0) quarters interleaved with wq1-3, then
        # the rest. Keeps the first QKV chain start at ~6us.
        w_dma(wq_sb[0], wqT, 0)
        x0_tiles = issue_x(0, 0)
        for j in range(1, QH):
            w_dma(wq_sb[j], wqT, j)
        w_dma(wk_sb, wkT, 0)
        w_dma(wv_sb, wvT, 0)
        nc.sync.dma_start(cos_sb[:], cosIn)
        nc.sync.dma_start(sin_sb[:], ssinIn)
        nc.sync.dma_start(mask_sb[:], masksIn.rearrange("p (r t) -> p r t", r=4))
        nc.sync.dma_start(id_sb[:], identIn)
        nc.sync.dma_start(ones_sb[:], onesIn)
        nc.sync.dma_start(wo_sb[:], woT.rearrange("(ko p) m -> p ko m", p=P))

        def rope(dst, src, n, tag):
            """dst = src*cos + rotate_half(src)*sin for token block n.

            dst: [P, TQ] bf16 AP; src: [P, TQ] fp32 AP (PSUM ok).
            ssin table is pre-negated on its top half."""
            c = cos_sb[:, n * TQ:(n + 1) * TQ]
            s = sin_sb[:, n * TQ:(n + 1) * TQ]
            rt = stage.tile([P, TQ], f32, tag="rt", name=f"rt_{n}_{tag}")
            t2 = stage.tile([P, TQ], f32, tag="rt2", name=f"r2_{n}_{tag}")
            nc.vector.tensor_copy(rt[0:64, :], src[64:P, :])
            nc.vector.tensor_copy(rt[64:P, :], src[0:64, :])
            nc.vector.tensor_tensor(rt[:], rt[:], s, OP.mult)
            nc.vector.tensor_tensor(t2[:], src, c, OP.mult)
            nc.vector.tensor_tensor(dst, t2[:], rt[:], OP.add)

        # ---------- output-projection work feeder ----------
        # Chunk c's projection = 2 halves x (4 k-groups x 2 m-tiles x 8 k) of
        # bf16 matmuls + 4 evictions, emitted between attention/QKV steps of
        # block c+2 so the PE queue always has independent, *ready* work
        # (the chunk's gather landed a full block earlier). at-loads ride
        # the vector queue: their data is always ready, so they never
        # head-of-line-block it (sync carries the x prefetches, which wait).
        def feeder_load(c):
            units = []
            for half in range(2):
                for g in range(4):
                    units.append(("dma", half, g))
                    for m in (half * 2, half * 2 + 1):
                        for k8 in range(8):
                            units.append(("mm", half, g, m, k8))
                units.append(("evict", half * 2))
                units.append(("evict", half * 2 + 1))
            return {"c": c, "units": units, "pos": 0, "at": {}, "ps": {}}

        def feeder_load_tail(c):
            # per-head-gather variant: k visits head-major (k = c'*4 + h) so
            # each section only needs gather h; PSUM accumulation order-free
            units = []
            for half in range(2):
                for h in range(QH):
                    units.append(("tdma", half, h))
                    for m in (half * 2, half * 2 + 1):
                        for cc in range(NCORES):
                            units.append(("tmm", half, h, m, cc))
                units.append(("evict", half * 2))
                units.append(("evict", half * 2 + 1))
            return {"c": c, "units": units, "pos": 0, "at": {}, "ps": {}}

        def feeder_emit(fds, nu):
            st = None
            for cand in fds:
                if cand is not None and cand["pos"] < len(cand["units"]):
                    st = cand
                    break
            if st is None:
                return
            c, units = st["c"], st["units"]
            for u in units[st["pos"]:st["pos"] + nu]:
                kind = u[0]
                if kind == "dma":
                    _, half, g = u
                    t = atpool.tile([P, 8, TQ], bf16, tag="at",
                                    name=f"at{c}_{half}_{g}")
                    nc.sync.dma_start(
                        t[:],
                        ag_out[c].rearrange("(ko p) t -> p ko t", p=P)[
                            :, g * 8:(g + 1) * 8, :])
                    st["at"][(half, g)] = t
                elif kind == "mm":
                    _, half, g, m, k8 = u
                    if g == 0 and k8 == 0:
                        st["ps"][m] = pop.tile([P, TQ], f32, tag=f"op{m % 2}",
                                               name=f"op{c}_{m}")
                    nc.tensor.matmul(
                        st["ps"][m][:], wo_sb[:, g * 8 + k8, m * P:(m + 1) * P],
                        st["at"][(half, g)][:, k8, :],
                        start=(g == 0 and k8 == 0), stop=(g == 3 and k8 == 7))
                elif kind == "tdma":
                    _, half, h = u
                    t = atpool.tile([P, 8, TQ], bf16, tag="at",
                                    name=f"att{half}_{h}")
                    nc.sync.dma_start(
                        t[:], agt_out[h].rearrange("(cp p) t -> p cp t", p=P))
                    st["at"][(half, h)] = t
                elif kind == "tmm":
                    _, half, h, m, cc = u
                    if h == 0 and cc == 0:
                        st["ps"][m] = pop.tile([P, TQ], f32, tag=f"op{m % 2}",
                                               name=f"opt_{m}")
                    nc.tensor.matmul(
                        st["ps"][m][:],
                        wo_sb[:, cc * QH + h, m * P:(m + 1) * P],
                        st["at"][(half, h)][:, cc, :],
                        start=(h == 0 and cc == 0),
                        stop=(h == QH - 1 and cc == NCORES - 1))
                elif kind == "pre":
                    u[1]()
                else:  # evict
                    _, m = u
                    ob = obpool.tile([P, TQ], f32, tag="ob", name=f"ob{c}_{m}")
                    nc.scalar.copy(ob[:], st["ps"][m][:])
                    nc.gpsimd.dma_start(
                        outT[m * P:(m + 1) * P, c * TQ:(c + 1) * TQ], ob[:])
            st["pos"] = min(st["pos"] + nu, len(units))

        def feeder_flush(fds):
            for st in fds:
                if st is not None:
                    feeder_emit([st], len(st["units"]))

        # ---------- main merged loop ----------
        feeders = [feeder_load(c) for c in range(NCHUNK - 1)]
        feeders.append(feeder_load_tail(NCHUNK - 1))
        for b in range(B):
            kT_cache = kvpool.tile([P, S], bf16, tag="kT")
            v_cache = kvpool.tile([P, S // P, HEAD_DIM], bf16, tag="v")
            for n in range(NB):
                ch = b * NB + n
                xq = x0_tiles if ch == 0 else issue_x(b, n)
                # block ch drains chunk ch-2 (its gather landed a block ago);
                # the last block additionally drains chunk 6.
                fds = [feeders[ch - 2]] if ch >= 2 else []
                if ch == NCHUNK - 1:
                    fds.append(feeders[ch - 1])

                # ---- QKV: six chains over the full contraction ----
                qT_sb = qpool.tile([P, QH, TQ], bf16, tag="q")
                chains = (
                    [(wq_sb[j], ("q", j)) for j in range(QH)]
                    + [(wk_sb, ("k",)), (wv_sb, ("v",))]
                )
                for ci, (w_t, what) in enumerate(chains):
                    ps = pq.tile([P, TQ], f32, tag="qkv",
                                 name=f"qkv{ch}_{what}")
                    for k in range(KT):
                        nc.tensor.matmul(
                            ps[:], w_t[:, k, :], xq[k // 8][:, k % 8, :],
                            start=(k == 0), stop=(k == KT - 1))
                    feeder_emit(fds, 1 if ci == 0 else (4 if ci >= 2 else 0))
                    if what[0] == "q":
                        rope(qT_sb[:, what[1], :], ps[:], n, f"q{what[1]}")
                    elif what[0] == "k":
                        rope(kT_cache[:, n * TQ:(n + 1) * TQ], ps[:], n, "k")
                    else:
                        vT_sb = stage.tile([P, TQ], f32, tag="vt",
                                           name=f"vt{ch}")
                        nc.scalar.copy(vT_sb[:], ps[:])
                        for i in range(4):
                            tp = pst.tile([P, TQ], f32, tag="st",
                                          name=f"vtr{ch}_{i}")
                            nc.tensor.transpose(
                                tp[:, :P], vT_sb[:, i * P:(i + 1) * P],
                                id_sb[:])
                            nc.vector.tensor_copy(
                                v_cache[:, n * 4 + i, :], tp[:, :P])

                # ---- attention; pv(t-1) issued after scores(t) so the PE
                # never waits on the exp of the tile it just produced.
                # The last block keeps its in-attention drain rate low so
                # enough ready projection work remains to cover the final
                # chunk's gather latency after attention ends. ----
                rate = 1 if ch == NCHUNK - 1 else 2
                ntk = (n + 1) * (TQ // P)
                for h in range(QH):
                    acc = aux.tile([P, TQ], f32r, tag="acc", name=f"ac{ch}{h}")
                    pv_ps = ppv.tile([P, TQ], f32, tag="pv", name=f"pv{ch}{h}")
                    pend = None  # (es, c0, t) awaiting its pv matmul
                    for t in range(ntk):
                        dg = t - (ntk - 4)  # diagonal tile index, >=0 on diag
                        c0 = dg * P if dg > 0 else 0
                        st_ps = pst.tile([P, TQ], f32, tag="st",
                                         name=f"st{ch}_{h}_{t}")
                        nc.tensor.matmul(
                            st_ps[:, c0:], kT_cache[:, t * P:(t + 1) * P],
                            qT_sb[:, h, c0:], start=True, stop=True)
                        if pend is not None:
                            pes, pc0, pt = pend
                            nc.tensor.matmul(
                                pv_ps[:, pc0:], v_cache[:, pt, :],
                                pes[:, pc0:], start=(pt == 0), stop=False,
                                skip_group_check=True)
                        es = epool.tile([P, TQ], bf16, tag="es",
                                        name=f"es{ch}_{h}_{t}")
                        nc.scalar.activation(es[:, c0:], st_ps[:, c0:], AF.Exp)
                        if dg >= 0:
                            nc.vector.tensor_tensor(
                                es[:, c0:], es[:, c0:], mask_sb[:, dg, c0:],
                                OP.mult)
                        if t == 0:
                            nc.vector.tensor_copy(acc[:], es[:])
                        else:
                            nc.vector.tensor_tensor(
                                acc[:, c0:], acc[:, c0:].bitcast(f32),
                                es[:, c0:], OP.add)
                        pend = (es, c0, t)
                        feeder_emit(fds, rate)
                    pes, pc0, pt = pend
                    nc.tensor.matmul(
                        pv_ps[:, pc0:], v_cache[:, pt, :], pes[:, pc0:],
                        start=(pt == 0), stop=True, skip_group_check=True)
                    # denominator + normalize
                    dn_ps = pst.tile([P, TQ], f32, tag="st", name=f"dn{ch}{h}")
                    nc.tensor.matmul(dn_ps[:1, :], ones_sb[:], acc[:],
                                     start=True, stop=True)
                    rec = aux.tile([1, TQ], f32, tag="rec", name=f"rc{ch}{h}")
                    nc.vector.reciprocal_approx_fast(rec[:], dn_ps[:1, :])
                    bc = aux.tile([P, TQ], f32, tag="bc", name=f"bc{ch}{h}")
                    nc.gpsimd.partition_broadcast(bc[:], rec[:])
                    ao = aux.tile([P, TQ], bf16, tag="ao", name=f"ao{ch}{h}")
                    nc.vector.tensor_tensor(ao[:], pv_ps[:], bc[:], OP.mult)
                    if ch < NCHUNK - 1:
                        nc.gpsimd.dma_start(
                            ag_in[ch][h * P:(h + 1) * P, :], ao[:])
                    else:
                        nc.gpsimd.dma_start(agt_in[h][:, :], ao[:])
                        if not getattr(nc, "_skip_collectives", False):
                            nc.gpsimd.collective_compute(
                                "AllGather",
                                mybir.AluOpType.bypass,
                                replica_groups=[list(range(NCORES))],
                                ins=[agt_in[h].opt()],
                                outs=[agt_out[h].opt()],
                            )
                    feeder_emit(fds, 2)

                # ---- AllGather this chunk across the 8 cores ----
                if ch < NCHUNK - 1 and not getattr(
                        nc, "_skip_collectives", False):
                    nc.gpsimd.collective_compute(
                        "AllGather",
                        mybir.AluOpType.bypass,
                        replica_groups=[list(range(NCORES))],
                        ins=[ag_in[ch].opt()],
                        outs=[ag_out[ch].opt()],
                    )
                feeder_flush(fds)

        # tail: the last chunk's remaining projection work
        feeder_flush([feeders[NCHUNK - 1]])


_NC_CACHE = None


def _get_module():
    global _NC_CACHE
    if _NC_CACHE is None:
        _NC_CACHE = _build_module()
    return _NC_CACHE


def _host_consts():
    inv_freq = 1.0 / (ROPE_THETA ** (np.arange(0, HEAD_DIM, 2,
                                               dtype=np.float32) / HEAD_DIM))
    t = np.arange(S, dtype=np.float32)
    freqs = np.outer(t, inv_freq).astype(np.float32)      # [S, 64]
    cos_h = np.cos(freqs).T                               # [64, S]
    sin_h = np.sin(freqs).T
    cosT = np.ascontiguousarray(
        np.concatenate([cos_h, cos_h], axis=0)).astype(BF)
    ssinT = np.ascontiguousarray(
        np.concatenate([-sin_h, sin_h], axis=0)).astype(BF)

    i = np.arange(P)[:, None]
    j = np.arange(TQ)[None, :]
    masks = np.concatenate(
        [(i + r * P <= j).astype(np.float32) for r in range(4)], axis=1
    ).astype(BF)                                          # [128, 4*512]
    ident = np.eye(P, dtype=np.float32)
    ones = np.ones((P, 1), dtype=np.float32)
    return cosT, ssinT, masks, ident, ones


def make_in_maps(hidden_states, wq, wk, wv, wo):
    hidden_states = np.asarray(hidden_states, dtype=np.float32)
    wq = np.asarray(wq, dtype=np.float32)
    wk = np.asarray(wk, dtype=np.float32)
    wv = np.asarray(wv, dtype=np.float32)
    wo = np.asarray(wo, dtype=np.float32)

    xT = np.ascontiguousarray(
        hidden_states.reshape(TOK, HIDDEN).T).astype(BF)
    cosT, ssinT, masks, ident, ones = _host_consts()
    qscale = 1.0 / math.sqrt(HEAD_DIM)

    in_maps = []
    for c in range(NCORES):
        in_maps.append({
            "xT": xT,
            "wqT": np.ascontiguousarray(
                (wq[c * HG:(c + 1) * HG] * qscale).T).astype(BF),
            "wkT": np.ascontiguousarray(
                wk[c * HEAD_DIM:(c + 1) * HEAD_DIM].T).astype(BF),
            "wvT": np.ascontiguousarray(
                wv[c * HEAD_DIM:(c + 1) * HEAD_DIM].T).astype(BF),
            "woT": np.ascontiguousarray(wo[c * HG:(c + 1) * HG].T).astype(BF),
            "cosIn": cosT,
            "ssinIn": ssinT,
            "masksIn": masks,
            "identIn": ident,
            "onesIn": ones,
        })
    return in_maps


def assemble_output(results):
    out = np.empty((TOK, HIDDEN), dtype=np.float32)
    for c in range(NCORES):
        out[:, c * HG:(c + 1) * HG] = results[c]["outT"].T
    return out.reshape(B, S, HIDDEN)


def kernel(hidden_states, wq, wk, wv, wo):
    nc = _get_module()
    in_maps = make_in_maps(hidden_states, wq, wk, wv, wo)
    trace = bool(int(os.environ.get("KERNEL_TRACE", "0")))
    res = bass_utils.run_bass_kernel_spmd(
        nc, in_maps, core_ids=list(range(NCORES)), trace=trace
    )
    if trace:
        kernel.last_results = res
    return assemble_output(res.results)


kernel.last_results = None
